# revision 37
# baseline (speedup 1.0000x reference)
"""BAM-style attention block (avgpool8 -> 1024-token attention -> nearest-upsample + residual)
as a distributed Bass kernel on 8 TRN2 NeuronCores.

Sharding: core = b*2 + half  (b = batch 0..3, half = H-half 0..1).
Each core:
  phase 1: streams its x shard [512, 128, 256] per 128-channel group, loads
           alternating between the sync and scalar HWDGE rings, avg-pools 8x8
           on DVE, and pipelines a pairwise exchange of each pooled group
           (gpsimd ring) with the streaming. The LAST channel group's exchange
           is split into two token-half collectives so the first half lands
           ~24us before the stream ends.
  phase 2: q/k/v projections + 512x1024 attention (bf16) in local-first token
           order, software-pipelined against the stream: q/k/v accumulate
           per-cg partials as each group lands; the token-A half of the late
           work runs while the token-B exchange is still in flight; softmax
           normalization is deferred to a row-sum rescale of y (broadcast
           first on PE, reciprocal on ACT).
  phase 3: re-streams x in 1MB chunks (loads alternate sync/scalar; the first
           10 chunks ride sync so the whole pool can prefetch through the
           attention tail), adds the upsampled attention output on DVE into
           bf16 staging, writes out in bf16 (half the store bytes; host casts
           back to f32 - the 2e-2 rel-err budget dwarfs bf16 rounding).
"""

import os
import numpy as np

B, C, H, W = 4, 512, 256, 256
DS = 8
HL = H // 2            # 128 rows per core
IL = HL // DS          # 16 pooled rows per core
WP = W // DS           # 32 pooled cols
NLOC = IL * WP         # 512 local tokens
NH = NLOC // 2         # 256 tokens per exchange half
N = 2 * NLOC           # 1024 tokens
K = C // 8             # 64
CG = C // 128          # 4 channel groups
NT = N // 128          # 8 token tiles (0..3 local, 4..7 remote)

_CACHE = {}
TRACE = bool(int(os.environ.get("BAM_TRACE", "0")))
LAST_EXEC_NS = None
LAST_RESULT = None


def _build():
    import concourse.bass as bass
    import concourse.tile as tile
    from concourse import bacc, mybir
    from concourse.masks import make_identity

    f32 = mybir.dt.float32
    bf16 = mybir.dt.bfloat16
    ADD = mybir.AluOpType.add
    SUB = mybir.AluOpType.subtract
    MUL = mybir.AluOpType.mult
    AXY = mybir.AxisListType.XY
    Exp = mybir.ActivationFunctionType.Exp
    Rcp = mybir.ActivationFunctionType.Reciprocal
    POOL_SCALE = 1.0 / (DS * DS)
    GROUPS = [[0, 1], [2, 3], [4, 5], [6, 7]]

    nc = bacc.Bacc("TRN2", target_bir_lowering=False, debug=False, num_devices=8)

    x_ext = nc.dram_tensor("x", [C, HL, W], f32, kind="ExternalInput")
    wq_ext = nc.dram_tensor("wq", [K, C], f32, kind="ExternalInput")
    bq_ext = nc.dram_tensor("bq", [1, K], f32, kind="ExternalInput")
    wk_ext = nc.dram_tensor("wk", [K, C], f32, kind="ExternalInput")
    bk_ext = nc.dram_tensor("bk", [1, K], f32, kind="ExternalInput")
    wv_ext = nc.dram_tensor("wv", [C, C], f32, kind="ExternalInput")
    bv_ext = nc.dram_tensor("bv", [1, C], f32, kind="ExternalInput")
    out_ext = nc.dram_tensor("out", [C, HL, W], bf16, kind="ExternalOutput")

    with tile.TileContext(nc) as tc:
        with tc.tile_pool(name="persist", bufs=1) as persist, \
             tc.tile_pool(name="scratch", bufs=2) as scratch, \
             tc.tile_pool(name="p1", bufs=4) as p1, \
             tc.tile_pool(name="p3", bufs=9) as p3, \
             tc.tile_pool(name="p3b", bufs=7) as p3b, \
             tc.tile_pool(name="psA", bufs=4, space="PSUM") as psA, \
             tc.tile_pool(name="psY", bufs=1, space="PSUM") as psY, \
             tc.tile_pool(name="dram", bufs=1, space="DRAM") as dram:

            # ---- constants & weights (gpsimd ring; both HWDGE rings stream x) ----
            ident = persist.tile([128, 128], bf16, tag="ident")
            make_identity(nc, ident[:])
            ones = persist.tile([1, N], bf16, tag="ones")
            nc.vector.memset(ones[:], 1.0)
            ones_col = persist.tile([128, 1], bf16, tag="ones_col")
            nc.vector.memset(ones_col[:], 1.0)

            def load_bias(ext, n):
                st = scratch.tile([1, n], f32, tag="bstage")
                nc.gpsimd.dma_start(out=st[:], in_=ext.ap())
                bb = persist.tile([1, n], bf16, tag=f"b_{ext.name}", name=f"b_{ext.name}")
                nc.scalar.copy(out=bb[:], in_=st[:])
                return bb

            bq_b = load_bias(bq_ext, K)
            bk_b = load_bias(bk_ext, K)
            bv_b = load_bias(bv_ext, C)

            def load_qk_weight(ext):
                st = scratch.tile([K, C], f32, tag="wstage")
                nc.gpsimd.dma_start(out=st[:], in_=ext.ap())
                wb = persist.tile([K, C], bf16, tag=f"wb_{ext.name}", name=f"wb_{ext.name}")
                nc.scalar.copy(out=wb[:], in_=st[:])
                wT = []
                for cg in range(CG):
                    ps = psA.tile([128, K], bf16, tag="s")
                    nc.tensor.transpose(ps[:], wb[:, cg * 128:(cg + 1) * 128],
                                        ident[0:K, 0:K])
                    t = persist.tile([128, K], bf16, tag=f"wT_{ext.name}{cg}",
                                     name=f"wT_{ext.name}{cg}")
                    nc.scalar.copy(out=t[:], in_=ps[:])
                    wT.append(t)
                return wT

            wqT = load_qk_weight(wq_ext)
            wkT = load_qk_weight(wk_ext)

            # wvT[cg][c_loc, d] = Wv[d, cg*128 + c_loc]
            wvT = [persist.tile([128, C], bf16, tag=f"wvT{cg}", name=f"wvT{cg}")
                   for cg in range(CG)]
            for dt in range(CG):
                st = scratch.tile([128, C], f32, tag="wstage")
                nc.gpsimd.dma_start(out=st[:], in_=wv_ext.ap()[dt * 128:(dt + 1) * 128, :])
                wvb = scratch.tile([128, C], bf16, tag="wvstage")
                nc.scalar.copy(out=wvb[:], in_=st[:])
                for cg in range(CG):
                    ps = psA.tile([128, 128], bf16, tag="s")
                    nc.tensor.transpose(ps[:], wvb[:, cg * 128:(cg + 1) * 128], ident[:])
                    nc.scalar.copy(out=wvT[cg][:, dt * 128:(dt + 1) * 128], in_=ps[:])

            # ---- phase 1: stream x + avg-pool; per-cg exchange on the gpsimd ring ----
            # Tokens are kept LOCAL-FIRST through phase 2: columns [0:512] are this
            # core's tokens, [512:1024] the partner's. Softmax and the final
            # contraction are permutation-invariant over n, so the global order is
            # never materialized.
            xf = [persist.tile([128, NLOC], f32, tag=f"xf{cg}", name=f"xf{cg}")
                  for cg in range(CG)]
            xfb_loc = [persist.tile([128, NLOC], bf16, tag=f"xfl{cg}", name=f"xfl{cg}")
                       for cg in range(CG)]
            xfb_rem = [persist.tile([128, NLOC], bf16, tag=f"xfr{cg}", name=f"xfr{cg}")
                       for cg in range(CG)]
            # the exchange moves PRE-SCALED bf16 features; cg0..2 exchange whole
            # groups, cg3 exchanges token halves A/B so A lands mid-stream
            xf_loc_d = dram.tile([CG - 1, 128, NLOC], bf16, tag="xf_loc")
            xf_all_d = dram.tile([CG - 1, 2, 128, NLOC], bf16, tag="xf_all")
            xg3_in = [dram.tile([128, NH], bf16, tag=f"xg3i{h}", name=f"xg3i{h}")
                      for h in range(2)]
            xg3_out = [dram.tile([2, 128, NH], bf16, tag=f"xg3o{h}", name=f"xg3o{h}")
                       for h in range(2)]

            # biases are accumulated FIRST (start=True) into every PSUM
            # accumulator so the last in-stream partial completes the sum with
            # no extra bias hop on the critical path
            q_ps = psA.tile([K, NLOC], f32, tag="s")
            nc.tensor.matmul(q_ps[:], bq_b[:], ones[:, :NLOC], start=True, stop=False)
            kl_ps = psA.tile([K, NLOC], f32, tag="s")
            nc.tensor.matmul(kl_ps[:], bk_b[:], ones[:, :NLOC], start=True, stop=False)
            kr_ps = psA.tile([K, NLOC], f32, tag="s")
            nc.tensor.matmul(kr_ps[:], bk_b[:], ones[:, :NLOC], start=True, stop=False)

            # late v-tiles (local/remote token-B halves) build up per-cg in PSUM
            # banks borrowed from the (later) y accumulators
            vB_ps = [psY.tile([128, C], f32, tag=f"y{k}", name=f"vB{k}")
                     for k in range(4)]
            VB_NT = {2: 0, 3: 1, 6: 2, 7: 3}   # token tile -> vB_ps index
            for k in range(4):
                nc.tensor.matmul(vB_ps[k][:], ones[:, :128], bv_b[:],
                                 start=True, stop=False)

            def vb_partial(nt, cg, src, stop):
                j = nt % 4
                nc.tensor.matmul(vB_ps[VB_NT[nt]][:], src[:, j * 128:(j + 1) * 128],
                                 wvT[cg][:], start=False, stop=stop)

            def remote_recover(cg):
                # partner half = (h0 + h1) - local, recovered rank-agnostically.
                # On DVE; for cg < 3 the collective is long done when program
                # order reaches here.
                xfg = scratch.tile([128, N], bf16, tag="xfg", name=f"xfg{cg}")
                for hf in range(2):
                    nc.gpsimd.dma_start(out=xfg[:, hf * NLOC:(hf + 1) * NLOC],
                                        in_=xf_all_d[cg, hf])
                hsum = scratch.tile([128, NLOC], bf16, tag="hsum", bufs=1,
                                    name=f"hsum{cg}")
                nc.vector.tensor_tensor(out=hsum[:], in0=xfg[:, :NLOC],
                                        in1=xfg[:, NLOC:], op=ADD)
                nc.vector.tensor_tensor(out=xfb_rem[cg][:], in0=hsum[:],
                                        in1=xfb_loc[cg][:], op=SUB)
                nc.tensor.matmul(kr_ps[:], wkT[cg][:], xfb_rem[cg][:],
                                 start=False, stop=False)
                # remote token-B v partials for this group
                for nt in (6, 7):
                    vb_partial(nt, cg, xfb_rem[cg][:], stop=False)

            for cg in range(CG):
                for ib in range(IL):
                    x1 = p1.tile([128, DS, W], f32, tag="x1")
                    ring = nc.sync if (cg * IL + ib) % 2 == 0 else nc.scalar
                    ring.dma_start(
                        out=x1[:],
                        in_=x_ext.ap()[cg * 128:(cg + 1) * 128,
                                       ib * DS:(ib + 1) * DS, :])
                    nc.vector.tensor_reduce(
                        out=xf[cg][:, ib * WP:(ib + 1) * WP],
                        in_=x1[:].rearrange("p h (j z) -> p j h z", z=DS),
                        axis=AXY, op=ADD)
                    # scale+cast each completed quarter and stage it so the
                    # exchange fires the moment the last slice lands
                    if ib % 4 == 3:
                        qs = slice((ib - 3) * WP, (ib + 1) * WP)
                        nc.vector.tensor_scalar_mul(xfb_loc[cg][:, qs],
                                                    xf[cg][:, qs], POOL_SCALE)
                        if cg < CG - 1:
                            nc.gpsimd.dma_start(out=xf_loc_d[cg][:, qs],
                                                in_=xfb_loc[cg][:, qs])
                        else:
                            hf, off = divmod((ib - 3) * WP, NH)
                            nc.gpsimd.dma_start(
                                out=xg3_in[hf][:, off:off + 4 * WP],
                                in_=xfb_loc[cg][:, qs])
                            if ib % 8 == 7:
                                nc.gpsimd.collective_compute(
                                    "AllGather", mybir.AluOpType.bypass,
                                    ins=[xg3_in[hf].opt()],
                                    outs=[xg3_out[hf].opt()],
                                    replica_groups=GROUPS)

                # local q/k partials + local token-B v partials (overlap stream)
                last = cg == CG - 1
                nc.tensor.matmul(q_ps[:], wqT[cg][:], xfb_loc[cg][:],
                                 start=False, stop=last)
                nc.tensor.matmul(kl_ps[:], wkT[cg][:], xfb_loc[cg][:],
                                 start=False, stop=last)
                for nt in (2, 3):
                    vb_partial(nt, cg, xfb_loc[cg][:], stop=last)
                if cg < CG - 1:
                    nc.gpsimd.collective_compute(
                        "AllGather", mybir.AluOpType.bypass,
                        ins=[xf_loc_d[cg].opt()],
                        outs=[xf_all_d[cg].opt()],
                        replica_groups=GROUPS)
                if 0 < cg:
                    remote_recover(cg - 1)

            # ================= stream done; pipelined tail =================
            # 1. q / k_local complete (bias was accumulated first): just cast
            q_sb = persist.tile([K, NLOC], bf16, tag="q_sb")
            nc.vector.tensor_copy(out=q_sb[:], in_=q_ps[:])
            k_loc = persist.tile([K, NLOC], bf16, tag="k_loc")
            nc.vector.tensor_copy(out=k_loc[:], in_=kl_ps[:])

            vT = [persist.tile([128, C], bf16, tag=f"vT{nt}", name=f"vT{nt}")
                  for nt in range(NT)]
            for nt in (2, 3):
                nc.vector.tensor_copy(out=vT[nt][:], in_=vB_ps[VB_NT[nt]][:])

            # 2. token-A half of cg3's exchange landed mid-stream: recover it,
            #    finish k_remote[:, :NH]
            k_rem = persist.tile([K, NLOC], bf16, tag="k_rem")

            def recover3(hf):
                sl = slice(hf * NH, (hf + 1) * NH)
                xfg = scratch.tile([128, NLOC], bf16, tag="xfg3", name=f"xfg3{hf}")
                for p in range(2):
                    nc.gpsimd.dma_start(out=xfg[:, p * NH:(p + 1) * NH],
                                        in_=xg3_out[hf][p])
                hsum = scratch.tile([128, NH], bf16, tag="hsum3", name=f"hsum3{hf}")
                nc.vector.tensor_tensor(out=hsum[:], in0=xfg[:, :NH],
                                        in1=xfg[:, NH:], op=ADD)
                nc.vector.tensor_tensor(out=xfb_rem[CG - 1][:, sl], in0=hsum[:],
                                        in1=xfb_loc[CG - 1][:, sl], op=SUB)
                nc.tensor.matmul(kr_ps[:, sl], wkT[CG - 1][:],
                                 xfb_rem[CG - 1][:, sl], start=False, stop=True)
                nc.vector.tensor_copy(out=k_rem[:, sl], in_=kr_ps[:, sl])

            recover3(0)

            # 3. token-A v tiles (full chains, bias first; inputs all ready)
            def vt_tile_full(nt):
                src = xfb_loc if nt < 4 else xfb_rem
                j = nt % 4
                v_ps = psA.tile([128, C], f32, tag="s", name=f"v_ps{nt}")
                nc.tensor.matmul(v_ps[:], ones[:, :128], bv_b[:], start=True, stop=False)
                for cg in range(CG):
                    nc.tensor.matmul(v_ps[:], src[cg][:, j * 128:(j + 1) * 128],
                                     wvT[cg][:], start=False, stop=(cg == CG - 1))
                nc.vector.tensor_copy(out=vT[nt][:], in_=v_ps[:])

            for nt in (0, 1, 4, 5):
                vt_tile_full(nt)

            # 4. energies computed PRE-TRANSPOSED by swapping matmul operands:
            #    eT[n, m] = sum_k k[k, n] q[k, m], so exp() lands straight in
            #    attnT — no PE transposes, no row-major attn staging at all.
            #    (unnormalized exp(e/sqrt(K)); energies are tiny for this model
            #    so exp without max-subtraction is safe)
            attnT = [persist.tile([128, NLOC], bf16, tag=f"attnT{nt}", name=f"attnT{nt}")
                     for nt in range(NT)]

            def et_tile(nt):
                ksb = k_loc if nt < 4 else k_rem
                j = nt % 4
                eT_ps = psA.tile([128, NLOC], f32, tag="s", name=f"eT{nt}")
                nc.tensor.matmul(eT_ps[:], ksb[:, j * 128:(j + 1) * 128], q_sb[:],
                                 start=True, stop=True)
                nc.scalar.activation(out=attnT[nt][:], in_=eT_ps[:],
                                     func=Exp, scale=K ** -0.5)

            rs_ps = psA.tile([1, NLOC], f32, tag="s", name="rs_ps")
            for nt in (0, 1, 2, 3, 4, 5):
                et_tile(nt)
            for nt in (0, 1, 2, 3, 4, 5):
                nc.tensor.matmul(rs_ps[:], ones_col[:], attnT[nt][:],
                                 start=(nt == 0), stop=False)

            # 4b. start the y accumulation early for the two d-blocks whose
            #     PSUM banks (ex vB_ps of local token-B tiles) freed at stream
            #     end — this pulls 12 of the 32 y matmuls ahead of the
            #     exchange-B wait
            y_ps = [psY.tile([128, NLOC], f32, tag=f"y{dt}", name=f"yps{dt}")
                    for dt in range(CG)]
            for nt in range(6):
                for dt in (0, 1):
                    nc.tensor.matmul(y_ps[dt][:], vT[nt][:, dt * 128:(dt + 1) * 128],
                                     attnT[nt][:], start=(nt == 0), stop=False)

            # 5. token-B half: recover, finish the two late v tiles + energies
            recover3(1)
            for nt in (6, 7):
                vb_partial(nt, CG - 1, xfb_rem[CG - 1][:], stop=True)
                nc.vector.tensor_copy(out=vT[nt][:], in_=vB_ps[VB_NT[nt]][:])
            for nt in (6, 7):
                et_tile(nt)
            for nt in (6, 7):
                nc.tensor.matmul(rs_ps[:], ones_col[:], attnT[nt][:],
                                 start=False, stop=(nt == NT - 1))

            # 6. softmax denominators: broadcast the raw row-sums (bf16 PE
            #    matmul), then a [128, NLOC] reciprocal on DVE — emitted BEFORE
            #    the y matmul train so it overlaps it
            rs_bf = persist.tile([1, NLOC], bf16, tag="rs_bf")
            nc.vector.tensor_copy(out=rs_bf[:], in_=rs_ps[:])
            rb_ps = psA.tile([128, NLOC], f32, tag="s")
            nc.tensor.matmul(rb_ps[:], ones[:, 0:128], rs_bf[:], start=True, stop=True)
            rb_sb = persist.tile([128, NLOC], f32, tag="rb_sb")
            nc.vector.reciprocal(rb_sb[:], rb_ps[:])

            # 7. remaining y matmuls: late d-blocks (banks freed by the token-B
            #    vT casts) over all n, plus the early d-blocks' token-B tiles
            for nt in range(NT):
                for dt in (2, 3):
                    nc.tensor.matmul(y_ps[dt][:], vT[nt][:, dt * 128:(dt + 1) * 128],
                                     attnT[nt][:], start=(nt == 0),
                                     stop=(nt == NT - 1))
            for nt in (6, 7):
                for dt in (0, 1):
                    nc.tensor.matmul(y_ps[dt][:], vT[nt][:, dt * 128:(dt + 1) * 128],
                                     attnT[nt][:], start=False,
                                     stop=(nt == NT - 1))

            y = [persist.tile([128, NLOC], f32, tag=f"y{dt}", name=f"y{dt}")
                 for dt in range(CG)]
            for dt in range(CG):
                nc.vector.tensor_tensor(out=y[dt][:], in0=y_ps[dt][:], in1=rb_sb[:],
                                        op=MUL)

            # ---- phase 3: out = x + upsample8(y), written in bf16 ----
            # The last 6 stores are deferred and emitted after every load so
            # they can ride the (by then idle) sync/scalar rings too — the
            # drain after the final load is not bound by the gpsimd ring alone.
            NCI = CG * IL
            DEFER = NCI - 5
            deferred = []
            for ci in range(NCI):
                cg, ib = divmod(ci, IL)
                x3 = p3.tile([128, DS, W], f32, tag="x3")
                ring = nc.sync if (ci < 10 or ci % 2 == 0) else nc.scalar
                ring.dma_start(
                    out=x3[:],
                    in_=x_ext.ap()[cg * 128:(cg + 1) * 128,
                                   ib * DS:(ib + 1) * DS, :])
                x3b = p3b.tile([128, DS, W], bf16, tag="x3b")
                xv = x3[:].rearrange("p h (j z) -> p h j z", z=DS)
                ov = x3b[:].rearrange("p h (j z) -> p h j z", z=DS)
                yv = y[cg][:, ib * WP:(ib + 1) * WP] \
                    [:, None, :, None].broadcast_to([128, DS, WP, DS])
                nc.vector.tensor_tensor(out=ov, in0=xv, in1=yv, op=ADD)
                if ci < DEFER:
                    nc.gpsimd.dma_start(
                        out=out_ext.ap()[cg * 128:(cg + 1) * 128,
                                         ib * DS:(ib + 1) * DS, :],
                        in_=x3b[:])
                else:
                    deferred.append((cg, ib, x3b))
            for di, (cg, ib, x3b) in enumerate(deferred):
                ring = [nc.sync, nc.scalar, nc.gpsimd][di % 3]
                ring.dma_start(
                    out=out_ext.ap()[cg * 128:(cg + 1) * 128,
                                     ib * DS:(ib + 1) * DS, :],
                    in_=x3b[:])

    nc.finalize()
    return nc


def _get_nc():
    if "nc" not in _CACHE:
        _CACHE["nc"] = _build()
    return _CACHE["nc"]


def kernel(x, Wq, bq, Wk, bk, Wv, bv):
    global LAST_EXEC_NS, LAST_RESULT
    from concourse.bass_utils import run_bass_kernel_spmd

    x = np.asarray(x, dtype=np.float32)
    Wq = np.asarray(Wq, dtype=np.float32)
    bq = np.asarray(bq, dtype=np.float32).reshape(1, K)
    Wk = np.asarray(Wk, dtype=np.float32)
    bk = np.asarray(bk, dtype=np.float32).reshape(1, K)
    Wv = np.asarray(Wv, dtype=np.float32)
    bv = np.asarray(bv, dtype=np.float32).reshape(1, C)

    nc = _get_nc()
    in_maps = []
    for core in range(8):
        b, half = core // 2, core % 2
        in_maps.append({
            "x": np.ascontiguousarray(x[b, :, half * HL:(half + 1) * HL, :]),
            "wq": Wq, "bq": bq, "wk": Wk, "bk": bk, "wv": Wv, "bv": bv,
        })

    res = run_bass_kernel_spmd(nc, in_maps, core_ids=list(range(8)), trace=TRACE)
    LAST_EXEC_NS = res.exec_time_ns
    LAST_RESULT = res

    out = np.empty((B, C, H, W), dtype=np.float32)
    for core in range(8):
        b, half = core // 2, core % 2
        out[b, :, half * HL:(half + 1) * HL, :] = \
            np.asarray(res.results[core]["out"]).astype(np.float32)
    return out
